# revision 5
# baseline (speedup 1.0000x reference)
"""Trainium2 Bass kernel for nn_MultiHeadAttention_77232101917088.

Causal MHA where only the LAST token's projected output is returned:
    out = (softmax_causal(q k^T / sqrt(hd)) v)[:, -1, :] @ Wo + bo

Because only the last query row survives, the whole problem collapses
algebraically (last causal row attends to every position):
    q_last[b,:]   = x[b,-1,:] @ Wq
    u[b,h,d]      = sum_e Wk[d, h*128+e] * q_last[b, h*128+e]
    scores[b,j,h] = sum_d x[b,j,d] * u[b,h,d]          (no K/V materialization)
    p             = softmax_j(scores / sqrt(hd))
    w[b,h,d]      = sum_j p[b,h,j] * x[b,j,d]
    ctx[b, h*128:+128] = w[b,h,:] @ Wv[:, h*128:+128]
    out           = ctx @ Wo + bo

Sharding: model dim d=2048 is split into 8 chunks of 256 (one per core).
Per-core work: q column-shard -> AllGather(q, 16 KB) -> per-head u on the
local d-chunk -> partial scores -> AllReduce(scores, 256 KB) -> redundant
softmax on every core -> weighted sum w (local d-chunk) -> partial ctx ->
AllReduce(ctx, 16 KB) -> output column shard + bias.  Host only slices /
transposes inputs and concatenates the 8 output shards.
"""

import numpy as np

import concourse.bacc as bacc
import concourse.bass as bass
import concourse.bass_isa as bass_isa
import concourse.mybir as mybir
import concourse.tile as tile
from concourse.bass_utils import run_bass_kernel_spmd

P = 128          # partitions
B = 2            # batch
S = 2048         # sequence length
D = 2048         # model dim
NH = 16          # heads
HD = 128         # head dim
NC = 8           # cores
CH = D // NC     # per-core model-dim chunk (256)
CT = CH // P     # chunk subtiles (2)
DT = D // P      # full-depth subtiles (16)
JT = S // P      # sequence subtiles (16)
ISCALE = 1.0 / np.sqrt(HD)

FP32 = mybir.dt.float32
DEBUG = False    # emit intermediate tensors as extra outputs


def _build_program():
    nc = bacc.Bacc(
        "TRN2",
        target_bir_lowering=False,
        debug=False,
        enable_asserts=False,
        num_devices=NC,
    )

    # ---- per-core DRAM inputs --------------------------------------------
    # xlastT[c, b] = x[b, -1, c]
    xlastT = nc.dram_tensor("xlastT", [D, B], FP32, kind="ExternalInput").ap()
    # wq[c, m]  = Wq[c, i*CH + m]                      (column shard)
    wq = nc.dram_tensor("wq", [D, CH], FP32, kind="ExternalInput").ap()
    # wkT[c, dd] = Wk[i*CH + dd, c]                    (row shard, transposed)
    wkT = nc.dram_tensor("wkT", [D, CH], FP32, kind="ExternalInput").ap()
    # xT[b, dd, j] = x[b, j, i*CH + dd]                (depth chunk, transposed)
    xT = nc.dram_tensor("xT", [B, CH, S], FP32, kind="ExternalInput").ap()
    # xn[b, j, dd] = x[b, j, i*CH + dd]                (depth chunk, natural)
    xn = nc.dram_tensor("xn", [B, S, CH], FP32, kind="ExternalInput").ap()
    # wv[dd, c] = Wv[i*CH + dd, c]                     (row shard)
    wv = nc.dram_tensor("wv", [CH, D], FP32, kind="ExternalInput").ap()
    # wo[c, m] = Wo[c, i*CH + m]                       (column shard)
    wo = nc.dram_tensor("wo", [D, CH], FP32, kind="ExternalInput").ap()
    # bo_sh[m] = bo[i*CH + m]
    bo_sh = nc.dram_tensor("bo_sh", [CH], FP32, kind="ExternalInput").ap()

    # ---- outputs ---------------------------------------------------------
    # outT[m, b] = out[b, i*CH + m]
    outT = nc.dram_tensor("outT", [CH, B], FP32, kind="ExternalOutput").ap()
    dbg = {}
    if DEBUG:
        dbg["qT_full"] = nc.dram_tensor("qT_full", [D, B], FP32, kind="ExternalOutput").ap()
        dbg["uT"] = nc.dram_tensor("uT", [P, CT, B * NH], FP32, kind="ExternalOutput").ap()
        dbg["psc"] = nc.dram_tensor("psc", [P, B, JT, NH], FP32, kind="ExternalOutput").ap()
        dbg["wn"] = nc.dram_tensor("wn", [P, CT, B, NH], FP32, kind="ExternalOutput").ap()
        dbg["ctx"] = nc.dram_tensor("ctx", [P, NH, B], FP32, kind="ExternalOutput").ap()

    with tile.TileContext(nc) as tc:
        with (
            tc.tile_pool(name="persist", bufs=1) as pp,
            tc.tile_pool(name="work", bufs=3) as wp,
            tc.tile_pool(name="psum", bufs=4, space="PSUM") as psp,
            tc.tile_pool(name="psum1", bufs=2, space="PSUM") as psp1,
            tc.tile_pool(name="dram", bufs=1, space="DRAM") as dp,
        ):
            # ---- persistent SBUF loads ----------------------------------
            xlastT_sb = pp.tile([P, DT, B], FP32, name="xlastT_sb")
            nc.sync.dma_start(xlastT_sb[:], xlastT.rearrange("(t p) b -> p t b", p=P))
            wq_sb = pp.tile([P, DT, CH], FP32, name="wq_sb")
            nc.sync.dma_start(wq_sb[:], wq.rearrange("(t p) m -> p t m", p=P))
            wkT_sb = pp.tile([P, DT, CH], FP32, name="wkT_sb")
            nc.sync.dma_start(wkT_sb[:], wkT.rearrange("(t p) d -> p t d", p=P))
            xT_sb = [pp.tile([P, CT, S], FP32, name=f"xT_sb{b}") for b in range(B)]
            for b in range(B):
                nc.sync.dma_start(xT_sb[b][:], xT[b].rearrange("(c p) j -> p c j", p=P))
            xn_sb = [pp.tile([P, JT, CH], FP32, name=f"xn_sb{b}") for b in range(B)]
            for b in range(B):
                nc.sync.dma_start(xn_sb[b][:], xn[b].rearrange("(t p) d -> p t d", p=P))
            wv_sb = pp.tile([P, CT, D], FP32, name="wv_sb")
            nc.sync.dma_start(wv_sb[:], wv.rearrange("(c p) d -> p c d", p=P))
            wo_sb = pp.tile([P, DT, CH], FP32, name="wo_sb")
            nc.sync.dma_start(wo_sb[:], wo.rearrange("(t p) m -> p t m", p=P))
            bo_sb = pp.tile([P, CT], FP32, name="bo_sb")
            nc.sync.dma_start(bo_sb[:], bo_sh.rearrange("(c p) -> p c", p=P))

            ones_sb = pp.tile([P, 1], FP32, name="ones_sb")
            nc.vector.memset(ones_sb[:], 1.0)

            # ---- A: q column shard  qT_sh[m, b] -------------------------
            q_sb = wp.tile([P, CT, B], FP32, name="q_sb")
            for ocb in range(CT):
                ps_q = psp1.tile([P, B], FP32, name="ps_q", tag="ps1")
                for t in range(DT):
                    nc.tensor.matmul(
                        ps_q[:],
                        lhsT=wq_sb[:, t, ocb * P:(ocb + 1) * P],
                        rhs=xlastT_sb[:, t, :],
                        start=(t == 0),
                        stop=(t == DT - 1),
                    )
                nc.any.tensor_copy(q_sb[:, ocb, :], ps_q[:])

            # ---- AllGather(q): [CH, B] per core -> [D, B] ---------------
            ag_in = dp.tile([CH, B], FP32, name="ag_in")
            ag_out = dp.tile([D, B], FP32, name="ag_out")
            nc.sync.dma_start(ag_in.rearrange("(c p) b -> p c b", p=P), q_sb[:])
            nc.gpsimd.collective_compute(
                "AllGather",
                mybir.AluOpType.bypass,
                replica_groups=[list(range(NC))],
                ins=[ag_in.opt()],
                outs=[ag_out.opt()],
            )
            qT_sb = wp.tile([P, DT, B], FP32, name="qT_sb")
            nc.sync.dma_start(qT_sb[:], ag_out.rearrange("(t p) b -> p t b", p=P))
            if DEBUG:
                nc.sync.dma_start(dbg["qT_full"].rearrange("(t p) b -> p t b", p=P), qT_sb[:])

            # scaled q, then scatter into per-head masked layout
            qs_sb = wp.tile([P, DT, B], FP32, name="qs_sb")
            nc.vector.tensor_scalar_mul(qs_sb[:], qT_sb[:], ISCALE)
            qtil_sb = wp.tile([P, DT, B * NH], FP32, name="qtil_sb")
            nc.vector.memset(qtil_sb[:], 0.0)
            for b in range(B):
                for h in range(NH):
                    nc.any.tensor_copy(
                        qtil_sb[:, h, b * NH + h:b * NH + h + 1],
                        qs_sb[:, h, b:b + 1],
                    )

            # ---- B: uT[dd, bh] = sum_c WkT[c, dd] * qtil[c, bh] ---------
            uT_sb = wp.tile([P, CT, B * NH], FP32, name="uT_sb")
            for ds in range(CT):
                ps_u = psp1.tile([P, B * NH], FP32, name="ps_u", tag="ps1")
                for t in range(DT):
                    nc.tensor.matmul(
                        ps_u[:],
                        lhsT=wkT_sb[:, t, ds * P:(ds + 1) * P],
                        rhs=qtil_sb[:, t, :],
                        start=(t == 0),
                        stop=(t == DT - 1),
                    )
                nc.any.tensor_copy(uT_sb[:, ds, :], ps_u[:])
            if DEBUG:
                nc.sync.dma_start(dbg["uT"][:], uT_sb[:])

            # ---- C: partial scores psc[j, (b, jt, h)] -------------------
            psc_sb = wp.tile([P, B, JT, NH], FP32, name="psc_sb")
            for b in range(B):
                for jt in range(JT):
                    ps_s = psp.tile([P, NH], FP32, name="ps_s", tag="ps")
                    for ds in range(CT):
                        nc.tensor.matmul(
                            ps_s[:],
                            lhsT=xT_sb[b][:, ds, jt * P:(jt + 1) * P],
                            rhs=uT_sb[:, ds, b * NH:(b + 1) * NH],
                            start=(ds == 0),
                            stop=(ds == CT - 1),
                        )
                    nc.any.tensor_copy(psc_sb[:, b, jt, :], ps_s[:])

            # ---- AllReduce(scores) --------------------------------------
            ar_in = dp.tile([P, B * JT * NH], FP32, name="ar_in")
            ar_out = dp.tile([P, B * JT * NH], FP32, name="ar_out")
            nc.sync.dma_start(ar_in.rearrange("p (b t h) -> p b t h", b=B, t=JT), psc_sb[:])
            nc.gpsimd.collective_compute(
                "AllReduce",
                mybir.AluOpType.add,
                replica_groups=[list(range(NC))],
                ins=[ar_in.opt()],
                outs=[ar_out.opt()],
            )
            sc_sb = wp.tile([P, B, JT, NH], FP32, name="sc_sb")
            nc.sync.dma_start(sc_sb[:], ar_out.rearrange("p (b t h) -> p b t h", b=B, t=JT))
            if DEBUG:
                nc.sync.dma_start(dbg["psc"][:], sc_sb[:])

            # ---- D: softmax over j = (p, jt) per (b, h) -----------------
            # global max (single scalar across everything): free-reduce then
            # partition all-reduce; result identical on every core.
            fmax_sb = wp.tile([P, 1], FP32, name="fmax_sb")
            nc.vector.reduce_max(
                fmax_sb[:],
                sc_sb[:].rearrange("p b t h -> p (b t h)"),
                axis=mybir.AxisListType.X,
            )
            gmax_sb = wp.tile([P, 1], FP32, name="gmax_sb")
            nc.gpsimd.partition_all_reduce(
                gmax_sb[:], fmax_sb[:], channels=P, reduce_op=bass_isa.ReduceOp.max
            )
            negm_sb = wp.tile([P, 1], FP32, name="negm_sb")
            nc.vector.tensor_scalar_mul(negm_sb[:], gmax_sb[:], -1.0)

            e_sb = wp.tile([P, B, JT, NH], FP32, name="e_sb")
            nc.scalar.activation(
                e_sb[:], sc_sb[:], mybir.ActivationFunctionType.Exp,
                bias=negm_sb[:], scale=1.0,
            )

            # ---- E: w[dd, (b,h)] = sum_j e[j, b, h] * xn[j, dd] ---------
            w_sb = wp.tile([P, CT, B, NH], FP32, name="w_sb")
            for b in range(B):
                for ds in range(CT):
                    ps_w = psp.tile([P, NH], FP32, name="ps_w", tag="ps")
                    for jt in range(JT):
                        nc.tensor.matmul(
                            ps_w[:],
                            lhsT=xn_sb[b][:, jt, ds * P:(ds + 1) * P],
                            rhs=e_sb[:, b, jt, :],
                            start=(jt == 0),
                            stop=(jt == JT - 1),
                        )
                    nc.any.tensor_copy(w_sb[:, ds, b, :], ps_w[:])

            # z[b, h] = sum_j e
            z_sb = wp.tile([1, B, NH], FP32, name="z_sb")
            for b in range(B):
                ps_z = psp.tile([1, NH], FP32, name="ps_z", tag="ps")
                for jt in range(JT):
                    nc.tensor.matmul(
                        ps_z[:],
                        lhsT=ones_sb[:],
                        rhs=e_sb[:, b, jt, :],
                        start=(jt == 0),
                        stop=(jt == JT - 1),
                    )
                nc.any.tensor_copy(z_sb[:, b, :], ps_z[:])
            rz_sb = wp.tile([1, B, NH], FP32, name="rz_sb")
            nc.vector.reciprocal(rz_sb[:], z_sb[:])
            rzb_sb = wp.tile([P, B, NH], FP32, name="rzb_sb")
            nc.gpsimd.partition_broadcast(rzb_sb[:], rz_sb[:], channels=P)

            # w normalized: wn = w * rz  (broadcast over ds free dim)
            wn_sb = wp.tile([P, CT, B, NH], FP32, name="wn_sb")
            nc.vector.tensor_tensor(
                wn_sb[:],
                w_sb[:],
                rzb_sb[:, None, :, :].to_broadcast([P, CT, B, NH]),
                mybir.AluOpType.mult,
            )
            if DEBUG:
                nc.sync.dma_start(dbg["wn"][:], wn_sb[:])

            # ---- F: partial ctx^T[c, b] per head ------------------------
            ctxp_sb = wp.tile([P, NH, B], FP32, name="ctxp_sb")
            for h in range(NH):
                ps_c = psp.tile([P, B], FP32, name="ps_c", tag="ps")
                for ds in range(CT):
                    nc.tensor.matmul(
                        ps_c[:],
                        lhsT=wv_sb[:, ds, h * P:(h + 1) * P],
                        rhs=wn_sb[:, ds, :, h],
                        start=(ds == 0),
                        stop=(ds == CT - 1),
                    )
                nc.any.tensor_copy(ctxp_sb[:, h, :], ps_c[:])

            # ---- AllReduce(ctx) -----------------------------------------
            arc_in = dp.tile([P, NH * B], FP32, name="arc_in")
            arc_out = dp.tile([P, NH * B], FP32, name="arc_out")
            nc.sync.dma_start(arc_in.rearrange("p (h b) -> p h b", h=NH), ctxp_sb[:])
            nc.gpsimd.collective_compute(
                "AllReduce",
                mybir.AluOpType.add,
                replica_groups=[list(range(NC))],
                ins=[arc_in.opt()],
                outs=[arc_out.opt()],
            )
            ctx_sb = wp.tile([P, NH, B], FP32, name="ctx_sb")
            nc.sync.dma_start(ctx_sb[:], arc_out.rearrange("p (h b) -> p h b", h=NH))
            if DEBUG:
                nc.sync.dma_start(dbg["ctx"][:], ctx_sb[:])

            # ---- G: out column shard + bias -----------------------------
            out_sb = wp.tile([P, CT, B], FP32, name="out_sb")
            for ocb in range(CT):
                ps_o = psp1.tile([P, B], FP32, name="ps_o", tag="ps1")
                for t in range(DT):
                    nc.tensor.matmul(
                        ps_o[:],
                        lhsT=wo_sb[:, t, ocb * P:(ocb + 1) * P],
                        rhs=ctx_sb[:, t, :],
                        start=(t == 0),
                        stop=(t == DT - 1),
                    )
                nc.vector.tensor_tensor(
                    out_sb[:, ocb, :],
                    ps_o[:],
                    bo_sb[:, ocb, None].to_broadcast([P, B]),
                    mybir.AluOpType.add,
                )
            nc.sync.dma_start(outT.rearrange("(c p) b -> p c b", p=P), out_sb[:])

    nc.compile()
    return nc


_PROGRAM = None


def _get_program():
    global _PROGRAM
    if _PROGRAM is None:
        _PROGRAM = _build_program()
    return _PROGRAM


def _shard_inputs(x, Wq, Wk, Wv, Wo, bo):
    x = np.ascontiguousarray(x, dtype=np.float32)
    xlastT = np.ascontiguousarray(x[:, -1, :].T)          # [D, B]
    xTfull = np.ascontiguousarray(x.transpose(0, 2, 1))   # [B, D, S]
    in_maps = []
    for i in range(NC):
        sl = slice(i * CH, (i + 1) * CH)
        in_maps.append({
            "xlastT": xlastT,
            "wq": np.ascontiguousarray(Wq[:, sl]),
            "wkT": np.ascontiguousarray(Wk[sl, :].T),
            "xT": np.ascontiguousarray(xTfull[:, sl, :]),
            "xn": np.ascontiguousarray(x[:, :, sl]),
            "wv": np.ascontiguousarray(Wv[sl, :]),
            "wo": np.ascontiguousarray(Wo[:, sl]),
            "bo_sh": np.ascontiguousarray(bo[sl]),
        })
    return in_maps


def kernel(x, Wq, Wk, Wv, Wo, bo, _trace=False, _trace_cores=None):
    x = np.asarray(x, dtype=np.float32)
    Wq = np.asarray(Wq, dtype=np.float32)
    Wk = np.asarray(Wk, dtype=np.float32)
    Wv = np.asarray(Wv, dtype=np.float32)
    Wo = np.asarray(Wo, dtype=np.float32)
    bo = np.asarray(bo, dtype=np.float32)

    nc = _get_program()
    in_maps = _shard_inputs(x, Wq, Wk, Wv, Wo, bo)
    res = run_bass_kernel_spmd(
        nc, in_maps, core_ids=list(range(NC)),
        trace=_trace, trace_cores=_trace_cores,
    )
    out = np.empty((B, D), dtype=np.float32)
    for i in range(NC):
        out[:, i * CH:(i + 1) * CH] = res.results[i]["outT"].T
    if _trace:
        kernel._last_results = res
    return out


# revision 13
# speedup vs baseline: 1.0907x; 1.0907x over previous
"""Trainium2 Bass kernel for nn_MultiHeadAttention_77232101917088.

Causal MHA where only the LAST token's projected output is returned:
    out = (softmax_causal(q k^T / sqrt(hd)) v)[:, -1, :] @ Wo + bo

Because only the last query row survives, the whole problem collapses
algebraically (last causal row attends to every position):
    q_last[b,:]   = x[b,-1,:] @ Wq
    u[b,h,d]      = sum_e Wk[d, h*128+e] * q_last[b, h*128+e]
    scores[b,j,h] = sum_d x[b,j,d] * u[b,h,d]          (no K/V materialization)
    p             = softmax_j(scores / sqrt(hd))
    w[b,h,d]      = sum_j p[b,h,j] * x[b,j,d]
    ctx[b, h*128:+128] = w[b,h,:] @ Wv[:, h*128:+128]
    out           = ctx @ Wo + bo

Sharding: model dim d=2048 is split into 8 chunks of 256 (one per core).
Per-core work: q column-shard -> AllGather(q, 16 KB) -> per-head u on the
local d-chunk -> partial scores -> AllReduce(scores, 256 KB) -> redundant
softmax on every core -> weighted sum w (local d-chunk) -> partial ctx ->
AllReduce(ctx, 16 KB) -> output column shard + bias.  Host only slices /
transposes inputs and concatenates the 8 output shards.
"""

import numpy as np

import concourse.bacc as bacc
import concourse.bass as bass
import concourse.bass_isa as bass_isa
import concourse.mybir as mybir
import concourse.tile as tile
from concourse.masks import make_identity
from concourse.bass_utils import run_bass_kernel_spmd

P = 128          # partitions
B = 2            # batch
S = 2048         # sequence length
D = 2048         # model dim
NH = 16          # heads
HD = 128         # head dim
NC = 8           # cores
CH = D // NC     # per-core model-dim chunk (256)
CT = CH // P     # chunk subtiles (2)
DT = D // P      # full-depth subtiles (16)
JT = S // P      # sequence subtiles (16)
BH = B * NH      # 32
NJC = 4          # j chunks of 512 for score matmul
JC = S // NJC    # 512
ISCALE = 1.0 / np.sqrt(HD)

FP32 = mybir.dt.float32


def _build_program():
    nc = bacc.Bacc(
        "TRN2",
        target_bir_lowering=False,
        debug=False,
        enable_asserts=False,
        num_devices=NC,
    )

    # ---- per-core DRAM inputs --------------------------------------------
    xlastT = nc.dram_tensor("xlastT", [D, B], FP32, kind="ExternalInput").ap()
    wq = nc.dram_tensor("wq", [D, CH], FP32, kind="ExternalInput").ap()
    wkT = nc.dram_tensor("wkT", [D, CH], FP32, kind="ExternalInput").ap()
    xT = nc.dram_tensor("xT", [B, CH, S], FP32, kind="ExternalInput").ap()
    xn = nc.dram_tensor("xn", [B, S, CH], FP32, kind="ExternalInput").ap()
    wv = nc.dram_tensor("wv", [CH, D], FP32, kind="ExternalInput").ap()
    wo = nc.dram_tensor("wo", [D, CH], FP32, kind="ExternalInput").ap()
    bo_sh = nc.dram_tensor("bo_sh", [CH], FP32, kind="ExternalInput").ap()

    # out_sh[b, m] = out[b, i*CH + m]
    out_sh = nc.dram_tensor("out_sh", [B, CH], FP32, kind="ExternalOutput").ap()

    with tile.TileContext(nc) as tc:
        with (
            tc.tile_pool(name="persist", bufs=1) as pp,
            tc.tile_pool(name="work", bufs=1) as wp,
            tc.tile_pool(name="psum", bufs=4, space="PSUM") as psp,
            tc.tile_pool(name="psum1", bufs=2, space="PSUM") as psp1,
            tc.tile_pool(name="dram", bufs=1, space="DRAM") as dp,
        ):
            # ---- loads: critical-path order on sync; late loads on scalar
            xlastT_sb = pp.tile([P, DT, B], FP32, name="xlastT_sb")
            nc.sync.dma_start(xlastT_sb[:], xlastT.rearrange("(t p) b -> p t b", p=P))
            wq_sb = pp.tile([P, DT, CH], FP32, name="wq_sb")
            nc.sync.dma_start(wq_sb[:], wq.rearrange("(t p) m -> p t m", p=P))
            wkT_sb = pp.tile([P, DT, CH], FP32, name="wkT_sb")
            nc.sync.dma_start(wkT_sb[:], wkT.rearrange("(t p) d -> p t d", p=P))
            xT_sb = [pp.tile([P, CT, S], FP32, name=f"xT_sb{b}") for b in range(B)]
            for b in range(B):
                nc.sync.dma_start(xT_sb[b][:], xT[b].rearrange("(c p) j -> p c j", p=P))
            xn_sb = [pp.tile([P, JT, CH], FP32, name=f"xn_sb{b}") for b in range(B)]
            for b in range(B):
                nc.scalar.dma_start(xn_sb[b][:], xn[b].rearrange("(t p) d -> p t d", p=P))
            wv_sb = pp.tile([P, CT, D], FP32, name="wv_sb")
            nc.scalar.dma_start(wv_sb[:], wv.rearrange("(c p) d -> p c d", p=P))
            wo_sb = pp.tile([P, DT, CH], FP32, name="wo_sb")
            nc.scalar.dma_start(wo_sb[:], wo.rearrange("(t p) m -> p t m", p=P))
            bo_sb = pp.tile([1, CH], FP32, name="bo_sb")
            nc.scalar.dma_start(bo_sb[:], bo_sh.rearrange("(o m) -> o m", o=1))

            # ---- A: q shard, streaming form -----------------------------
            # psum [B, CH] = sum_t xlastT[:, t, :].T @ wq[:, t, :]
            ps_q = psp1.tile([B, CH], FP32, name="ps_q", tag="ps1")
            for t in range(DT):
                nc.tensor.matmul(
                    ps_q[:],
                    lhsT=xlastT_sb[:, t, :],
                    rhs=wq_sb[:, t, :],
                    start=(t == 0),
                    stop=(t == DT - 1),
                )
            q_sb = wp.tile([B, CH], FP32, name="q_sb")
            nc.any.tensor_copy(q_sb[:], ps_q[:])

            # ---- AllGather(q): [B, CH] per core -> [NC, B, CH] ----------
            ag_in = dp.tile([B, CH], FP32, name="ag_in")
            ag_out = dp.tile([NC, B, CH], FP32, name="ag_out")
            nc.gpsimd.dma_start(ag_in[:], q_sb[:])
            nc.gpsimd.collective_compute(
                "AllGather",
                mybir.AluOpType.bypass,
                replica_groups=[list(range(NC))],
                ins=[ag_in.opt()],
                outs=[ag_out.opt()],
            )
            # qT_sb[p, t, b] = q[b, t*128+p];  global col = r*CH + c*P + p
            qT_sb = wp.tile([P, DT, B], FP32, name="qT_sb")
            for b in range(B):
                for c in range(CT):
                    nc.gpsimd.dma_start(
                        qT_sb[:, :, b].rearrange("p (r c) -> p r c", r=NC)[:, :, c],
                        ag_out[:, b, c * P:(c + 1) * P].rearrange("r p -> p r"),
                    )

            # scaled q scattered into per-head masked layout
            qs_sb = wp.tile([P, DT, B], FP32, name="qs_sb")
            nc.vector.tensor_scalar_mul(qs_sb[:], qT_sb[:], ISCALE)
            qtil_sb = wp.tile([P, DT, BH], FP32, name="qtil_sb")
            nc.vector.memset(qtil_sb[:], 0.0)
            for b in range(B):
                for h in range(NH):
                    nc.any.tensor_copy(
                        qtil_sb[:, h, b * NH + h:b * NH + h + 1],
                        qs_sb[:, h, b:b + 1],
                    )

            # ---- B: uT[dd, bh] = sum_c WkT[c, dd] * qtil[c, bh] ---------
            uT_sb = wp.tile([P, CT, BH], FP32, name="uT_sb")
            for ds in range(CT):
                ps_u = psp1.tile([P, BH], FP32, name="ps_u", tag="ps1")
                for t in range(DT):
                    nc.tensor.matmul(
                        ps_u[:],
                        lhsT=wkT_sb[:, t, ds * P:(ds + 1) * P],
                        rhs=qtil_sb[:, t, :],
                        start=(t == 0),
                        stop=(t == DT - 1),
                    )
                nc.any.tensor_copy(uT_sb[:, ds, :], ps_u[:])

            # ---- C: partial scores, streaming form ----------------------
            # per batch: psc_b[h, j] — stationary uT b-cols (16), moving xT
            # in 512-wide chunks, accumulated over the 2 d-subtiles.
            psc_b = [wp.tile([NH, S], FP32, name=f"psc_b{b}") for b in range(B)]
            for b in range(B):
                for jc in range(NJC):
                    ps_s = psp.tile([NH, JC], FP32, name="ps_s", tag="ps")
                    for ds in range(CT):
                        nc.tensor.matmul(
                            ps_s[:],
                            lhsT=uT_sb[:, ds, b * NH:(b + 1) * NH],
                            rhs=xT_sb[b][:, ds, jc * JC:(jc + 1) * JC],
                            start=(ds == 0),
                            stop=(ds == CT - 1),
                        )
                    nc.any.tensor_copy(
                        psc_b[b][:, jc * JC:(jc + 1) * JC], ps_s[:]
                    )

            # ---- AllReduce(scores) --------------------------------------
            ar_in = dp.tile([B, NH, S], FP32, name="ar_in")
            ar_out = dp.tile([B, NH, S], FP32, name="ar_out")
            for b in range(B):
                nc.gpsimd.dma_start(ar_in[b], psc_b[b][:])
            nc.gpsimd.collective_compute(
                "AllReduce",
                mybir.AluOpType.add,
                replica_groups=[list(range(NC))],
                ins=[ar_in.opt()],
                outs=[ar_out.opt()],
            )
            sc_b = [wp.tile([NH, S], FP32, name=f"sc_b{b}") for b in range(B)]
            for b in range(B):
                nc.gpsimd.dma_start(sc_b[b][:], ar_out[b])

            # ---- D: softmax per (b,h) row; z via accum_out --------------
            eT_sb = wp.tile([P, JT, B, NH], FP32, name="eT_sb")
            ident_sb = pp.tile([NH, NH], FP32, name="ident_sb")
            make_identity(nc, ident_sb[:])
            for b in range(B):
                m_sb = wp.tile([NH, 1], FP32, name="m_sb", tag="m")
                nc.vector.reduce_max(m_sb[:], sc_b[b][:], axis=mybir.AxisListType.X)
                negm_sb = wp.tile([NH, 1], FP32, name="negm_sb", tag="negm")
                nc.vector.tensor_scalar_mul(negm_sb[:], m_sb[:], -1.0)
                e_sb = wp.tile([NH, S], FP32, name="e_sb", tag="e")
                z_sb = wp.tile([NH, 1], FP32, name="z_sb", tag="z")
                nc.scalar.activation(
                    e_sb[:], sc_b[b][:], mybir.ActivationFunctionType.Exp,
                    bias=negm_sb[:], scale=1.0, accum_out=z_sb[:],
                )
                rz_sb = wp.tile([NH, 1], FP32, name="rz_sb", tag="rz")
                nc.vector.reciprocal(rz_sb[:], z_sb[:])
                nc.vector.tensor_scalar_mul(e_sb[:], e_sb[:], rz_sb[:])
                for jt in range(JT):
                    ps_t = psp.tile([P, NH], FP32, name="ps_t", tag="ps")
                    nc.tensor.transpose(
                        ps_t[:], e_sb[:, jt * P:(jt + 1) * P], ident_sb[:]
                    )
                    nc.any.tensor_copy(eT_sb[:, jt, b, :], ps_t[:])

            # ---- E: w[dd, bh] = sum_j eT[j, bh] * xn[j, dd] -------------
            w_sb = wp.tile([P, CT, B, NH], FP32, name="w_sb")
            for b in range(B):
                for ds in range(CT):
                    ps_w = psp.tile([P, NH], FP32, name="ps_w", tag="ps")
                    for jt in range(JT):
                        nc.tensor.matmul(
                            ps_w[:],
                            lhsT=xn_sb[b][:, jt, ds * P:(ds + 1) * P],
                            rhs=eT_sb[:, jt, b, :],
                            start=(jt == 0),
                            stop=(jt == JT - 1),
                        )
                    nc.any.tensor_copy(w_sb[:, ds, b, :], ps_w[:])

            # ---- F: partial ctx^T[c, b] per head ------------------------
            ctxp_sb = wp.tile([P, NH, B], FP32, name="ctxp_sb")
            for h in range(NH):
                ps_c = psp.tile([P, B], FP32, name="ps_c", tag="ps")
                for ds in range(CT):
                    nc.tensor.matmul(
                        ps_c[:],
                        lhsT=wv_sb[:, ds, h * P:(h + 1) * P],
                        rhs=w_sb[:, ds, :, h],
                        start=(ds == 0),
                        stop=(ds == CT - 1),
                    )
                nc.any.tensor_copy(ctxp_sb[:, h, :], ps_c[:])

            # ---- AllReduce(ctx) -----------------------------------------
            arc_in = dp.tile([P, NH * B], FP32, name="arc_in")
            arc_out = dp.tile([P, NH * B], FP32, name="arc_out")
            nc.gpsimd.dma_start(arc_in.rearrange("p (h b) -> p h b", h=NH), ctxp_sb[:])
            nc.gpsimd.collective_compute(
                "AllReduce",
                mybir.AluOpType.add,
                replica_groups=[list(range(NC))],
                ins=[arc_in.opt()],
                outs=[arc_out.opt()],
            )
            ctx_sb = wp.tile([P, DT, B], FP32, name="ctx_sb")
            nc.gpsimd.dma_start(ctx_sb[:], arc_out.rearrange("p (h b) -> p h b", h=NH))

            # ---- G: out shard (streaming) + bias ------------------------
            ps_o = psp1.tile([B, CH], FP32, name="ps_o", tag="ps1")
            for t in range(DT):
                nc.tensor.matmul(
                    ps_o[:],
                    lhsT=ctx_sb[:, t, :],
                    rhs=wo_sb[:, t, :],
                    start=(t == 0),
                    stop=(t == DT - 1),
                )
            bo2_sb = wp.tile([B, CH], FP32, name="bo2_sb")
            nc.gpsimd.partition_broadcast(bo2_sb[:], bo_sb[:], channels=B)
            o_sb = wp.tile([B, CH], FP32, name="o_sb")
            nc.vector.tensor_tensor(o_sb[:], ps_o[:], bo2_sb[:], mybir.AluOpType.add)
            nc.gpsimd.dma_start(out_sh[:], o_sb[:])

    nc.compile()
    return nc


_PROGRAM = None


def _get_program():
    global _PROGRAM
    if _PROGRAM is None:
        _PROGRAM = _build_program()
    return _PROGRAM


def _shard_inputs(x, Wq, Wk, Wv, Wo, bo):
    x = np.ascontiguousarray(x, dtype=np.float32)
    xlastT = np.ascontiguousarray(x[:, -1, :].T)          # [D, B]
    xTfull = np.ascontiguousarray(x.transpose(0, 2, 1))   # [B, D, S]
    in_maps = []
    for i in range(NC):
        sl = slice(i * CH, (i + 1) * CH)
        in_maps.append({
            "xlastT": xlastT,
            "wq": np.ascontiguousarray(Wq[:, sl]),
            "wkT": np.ascontiguousarray(Wk[sl, :].T),
            "xT": np.ascontiguousarray(xTfull[:, sl, :]),
            "xn": np.ascontiguousarray(x[:, :, sl]),
            "wv": np.ascontiguousarray(Wv[sl, :]),
            "wo": np.ascontiguousarray(Wo[:, sl]),
            "bo_sh": np.ascontiguousarray(bo[sl]),
        })
    return in_maps


def kernel(x, Wq, Wk, Wv, Wo, bo, _trace=False, _trace_cores=None):
    x = np.asarray(x, dtype=np.float32)
    Wq = np.asarray(Wq, dtype=np.float32)
    Wk = np.asarray(Wk, dtype=np.float32)
    Wv = np.asarray(Wv, dtype=np.float32)
    Wo = np.asarray(Wo, dtype=np.float32)
    bo = np.asarray(bo, dtype=np.float32)

    nc = _get_program()
    in_maps = _shard_inputs(x, Wq, Wk, Wv, Wo, bo)
    res = run_bass_kernel_spmd(
        nc, in_maps, core_ids=list(range(NC)),
        trace=_trace, trace_cores=_trace_cores,
    )
    out = np.empty((B, D), dtype=np.float32)
    for i in range(NC):
        out[:, i * CH:(i + 1) * CH] = res.results[i]["out_sh"]
    if _trace:
        kernel._last_results = res
    return out
